# revision 11
# baseline (speedup 1.0000x reference)
"""Two-layer GCN (PyG GCNConv semantics) on 8 Trainium2 NeuronCores.

Strategy (dst-sharded message passing, Ant dma_gather + windowed PE reduce):
  - Nodes padded to NPAD = 128*TBLK; core k owns dst nodes
    [k*PER_CORE, (k+1)*PER_CORE).
  - Normalization folded into node features: with g = dinv * (h @ W),
      out[d] = dinv[d] * (sum_{e: dst=d} g[src_e] + g[d]) + b
    so per-edge work is a pure gather + segment-sum (no per-edge scale),
    and self-loops become an elementwise term.
  - Per layer, each core:
      1. GEMM: full table g = dinv*(h@W) -> bf16 table in its HBM,
         rows padded to ELEM=128 bf16 (256 B) for dma_gather, p-major
         row order so table writes are big contiguous DMAs.
      2. Per chunk of edge-tiles: Ant dma_gather (one instr per
         (chunk, src-range); int16 indices local to a <=32768-row range)
         pulls per-edge rows into SBUF with edge slot -> partition i%128.
         DVE builds a one-hot window-membership from host window
         positions; PE matmuls membership^T[128e x W] @ msgs[128e x F]
         accumulate per-window (W consecutive dsts) sums in PSUM.
         No scatter-add: windows tile the dst space densely.
      3. Elementwise finale (self term, dinv scale, bias, relu).
  - Between layers: transpose own hidden block on PE, AllGather so every
    core can compute the full layer-2 table.
  - Edges sorted by (dst-window, src-range); each (core,window,range)
    edge list padded to a shared tile count (SPMD: one instruction
    stream, per-core data).
"""

import os
from contextlib import ExitStack

import numpy as np
import ml_dtypes

BF16 = ml_dtypes.bfloat16
P = 128
ELEM = 128          # table row width in bf16 elements (256 B)
NR = 4              # src index ranges (int16 gather indices)


class Cfg:
    def __init__(self, n_nodes, n_feat, hid1, hid2, tblk, ncores=8,
                 win=32, chunk_w=4):
        assert tblk % ncores == 0
        self.N, self.F0, self.F1, self.F2 = n_nodes, n_feat, hid1, hid2
        self.TBLK = tblk
        self.NCORES = ncores
        self.WIN = win
        self.CHUNK_W = chunk_w
        self.NPAD = P * tblk
        assert self.NPAD >= n_nodes + 1
        self.PER_CORE = self.NPAD // ncores
        self.TPC = self.PER_CORE // P
        self.WINDOWS = self.PER_CORE // win
        assert self.WINDOWS % chunk_w == 0
        self.N_CHUNKS = self.WINDOWS // chunk_w
        assert self.NPAD % NR == 0
        self.RS = self.NPAD // NR          # rows per gather range
        assert self.RS <= 32768
        assert self.TBLK % NR == 0

    def row_of(self, n):
        """Table row of node n (p-major: row = (n%128)*TBLK + n//128)."""
        return (n % P) * self.TBLK + n // P

    def node_of_row(self, r):
        return (r % self.TBLK) * P + r // self.TBLK


FULL = Cfg(n_nodes=100000, n_feat=50, hid1=30, hid2=25, tblk=784)


# ----------------------------------------------------------- host sharding --
def prepare(cfg, x, edge_index):
    """Build per-core device inputs + shared schedule (tiles_wg)."""
    src = np.asarray(edge_index[0], dtype=np.int64)
    dst = np.asarray(edge_index[1], dtype=np.int64)

    deg = np.bincount(dst, minlength=cfg.N).astype(np.float64) + 1.0
    dinv = (1.0 / np.sqrt(deg)).astype(np.float32)
    dinv_pad = np.zeros(cfg.NPAD, np.float32)
    dinv_pad[: cfg.N] = dinv

    rows = cfg.row_of(src)
    rng = rows // cfg.RS
    loc = (rows - rng * cfg.RS).astype(np.int16)

    core = dst // cfg.PER_CORE
    dloc = dst - core * cfg.PER_CORE
    w_id = dloc // cfg.WIN
    pos = (dloc % cfg.WIN).astype(np.float32)

    key = (core * cfg.WINDOWS + w_id) * NR + rng
    counts = np.bincount(key, minlength=cfg.NCORES * cfg.WINDOWS * NR).reshape(
        cfg.NCORES, cfg.WINDOWS, NR)
    tiles_wg = np.ceil(counts.max(axis=0) / P).astype(np.int64)  # [WINDOWS, NR]
    empty = tiles_wg.sum(axis=1) == 0
    tiles_wg[empty, 0] = 1
    t_total = int(tiles_wg.sum())

    # per-range zero row (a pad-node row inside the range)
    zloc = np.empty(NR, np.int16)
    for g in range(NR):
        offs = np.arange(cfg.RS)
        nodes = cfg.node_of_row(g * cfg.RS + offs)
        z = np.nonzero(nodes >= cfg.N)[0]
        assert len(z) > 0, "no zero row in range"
        zloc[g] = z[0]

    # tile offsets: chunk-major, within chunk range-major then window
    CW = cfg.CHUNK_W
    tile_off = np.zeros((cfg.WINDOWS, NR), np.int64)
    off = 0
    for c in range(cfg.N_CHUNKS):
        for g in range(NR):
            for wl in range(CW):
                w = c * CW + wl
                tile_off[w, g] = off
                off += tiles_wg[w, g]
    assert off == t_total

    order = np.argsort(key, kind="stable")
    loc_s = loc[order]
    pos_s = pos[order]
    starts = np.concatenate([[0], np.cumsum(counts.reshape(-1))])

    per_core = []
    for k in range(cfg.NCORES):
        es = np.empty(t_total * P, np.int16)
        dbg = np.empty(t_total * P, np.int64)
        ps = np.zeros(t_total * P, np.float32)
        # default pads: per-slot zero row of the slot's range
        for w in range(cfg.WINDOWS):
            for g in range(NR):
                o = tile_off[w, g] * P
                ntile = tiles_wg[w, g]
                if ntile == 0:
                    continue
                es[o : o + ntile * P] = zloc[g]
                dbg[o : o + ntile * P] = g * cfg.RS + int(zloc[g])
                i0 = starts[(k * cfg.WINDOWS + w) * NR + g]
                i1 = starts[(k * cfg.WINDOWS + w) * NR + g + 1]
                cnt = i1 - i0
                es[o : o + cnt] = loc_s[i0:i1]
                dbg[o : o + cnt] = g * cfg.RS + loc_s[i0:i1].astype(np.int64)
                ps[o : o + cnt] = pos_s[i0:i1]
        # wrapped int16 layout: position i -> [i%16, i//16], replicated to 128
        wrap = np.ascontiguousarray(
            np.tile(es.reshape(t_total * 8, 16).T, (8, 1)))   # [128, 8*t_total]
        dpos_all = np.ascontiguousarray(
            ps.reshape(t_total, P).T).astype(BF16)            # [128, t_total]
        sl = slice(k * cfg.PER_CORE, (k + 1) * cfg.PER_CORE)
        dinv_own = np.ascontiguousarray(dinv_pad[sl].reshape(cfg.TPC, P).T)
        per_core.append(dict(idx_all=wrap, dpos_all=dpos_all,
                             dinv_own=dinv_own, dbg_rows=dbg))

    dinv_gemm = np.ascontiguousarray(dinv_pad.reshape(cfg.TBLK, P).T)
    xT = np.zeros((cfg.F0, cfg.NPAD), BF16)
    xT[:, : cfg.N] = np.asarray(x, np.float32).T.astype(BF16)
    iota = np.broadcast_to(
        np.arange(cfg.WIN, dtype=np.float32).astype(BF16), (P, cfg.WIN))
    ident = np.eye(P, dtype=np.float32)
    shared = dict(xT=xT, dinv_gemm=dinv_gemm,
                  iota=np.ascontiguousarray(iota), ident=ident)
    for k in range(cfg.NCORES):
        per_core[k]["xT_own"] = np.ascontiguousarray(
            xT[:, k * cfg.PER_CORE : (k + 1) * cfg.PER_CORE])
    return tiles_wg, per_core, shared


def unshard(cfg, outs):
    full = np.empty((cfg.NPAD, cfg.F2), np.float32)
    for k, o in enumerate(outs):
        blk = o.reshape(P, cfg.TPC, cfg.F2).transpose(1, 0, 2)
        full[k * cfg.PER_CORE : (k + 1) * cfg.PER_CORE] = blk.reshape(
            cfg.PER_CORE, cfg.F2)
    return full[: cfg.N]


# ------------------------------------------------------------ device build --
def build(cfg, tiles_wg):
    import concourse.bass as bass
    import concourse.tile as tile
    from concourse import bacc, mybir

    dt = mybir.dt
    F0, F1, F2 = cfg.F0, cfg.F1, cfg.F2
    TBLK, TPC, WIN, CW = cfg.TBLK, cfg.TPC, cfg.WIN, cfg.CHUNK_W
    NPAD, PER_CORE, NCORES = cfg.NPAD, cfg.PER_CORE, cfg.NCORES
    t_total = int(np.sum(tiles_wg))
    chunk_tiles = [
        int(np.sum(tiles_wg[c * CW : (c + 1) * CW]))
        for c in range(cfg.N_CHUNKS)]
    t_chunk_max = max(chunk_tiles)
    SW = max(1, TPC // 4)      # staging tiles per table-write DMA

    nc = bacc.Bacc(
        "TRN2", target_bir_lowering=False, debug=False,
        enable_asserts=False, num_devices=NCORES,
    )

    xT_d = nc.dram_tensor("xT", [F0, NPAD], dt.bfloat16, kind="ExternalInput")
    xTo_d = nc.dram_tensor("xT_own", [F0, PER_CORE], dt.bfloat16,
                           kind="ExternalInput")
    w1_d = nc.dram_tensor("W1", [F0, F1], dt.bfloat16, kind="ExternalInput")
    b1_d = nc.dram_tensor("b1", [P, F1], dt.float32, kind="ExternalInput")
    w2_d = nc.dram_tensor("W2", [F1, F2], dt.bfloat16, kind="ExternalInput")
    b2_d = nc.dram_tensor("b2", [P, F2], dt.float32, kind="ExternalInput")
    dgm_d = nc.dram_tensor("dinv_gemm", [P, TBLK], dt.float32,
                           kind="ExternalInput")
    dow_d = nc.dram_tensor("dinv_own", [P, TPC], dt.float32,
                           kind="ExternalInput")
    idx_d = nc.dram_tensor("idx_all", [P, 8 * t_total], dt.int16,
                           kind="ExternalInput")
    dps_d = nc.dram_tensor("dpos_all", [P, t_total], dt.bfloat16,
                           kind="ExternalInput")
    iota_d = nc.dram_tensor("iota", [P, WIN], dt.bfloat16, kind="ExternalInput")
    id_d = nc.dram_tensor("ident", [P, P], dt.float32, kind="ExternalInput")
    out_d = nc.dram_tensor("out", [P, TPC * F2], dt.float32,
                           kind="ExternalOutput")

    table1 = nc.dram_tensor("table1", [NPAD, ELEM], dt.bfloat16,
                            kind="Internal")
    table2 = nc.dram_tensor("table2", [NPAD, ELEM], dt.bfloat16,
                            kind="Internal")
    h1T_own_d = nc.dram_tensor("h1T_own", [F1, PER_CORE], dt.bfloat16,
                               kind="Internal")
    h1T_all_d = nc.dram_tensor("h1T_all", [NCORES * F1, PER_CORE], dt.bfloat16,
                               kind="Internal", addr_space="Shared")

    with tile.TileContext(nc) as tc, ExitStack() as ctx:
        const = ctx.enter_context(tc.tile_pool(name="const", bufs=1))
        persist = ctx.enter_context(tc.tile_pool(name="persist", bufs=1))
        xslab = ctx.enter_context(tc.tile_pool(name="xslab", bufs=2))
        stgp = ctx.enter_context(tc.tile_pool(name="stg", bufs=2))
        idxp = ctx.enter_context(tc.tile_pool(name="idx", bufs=2))
        dposp = ctx.enter_context(tc.tile_pool(name="dpos", bufs=2))
        msgsp = ctx.enter_context(tc.tile_pool(name="msgs", bufs=2))
        membp = ctx.enter_context(tc.tile_pool(name="memb", bufs=2))
        gpsum = ctx.enter_context(tc.tile_pool(name="gpsum", bufs=2,
                                               space="PSUM"))
        wpsum = ctx.enter_context(tc.tile_pool(name="wpsum", bufs=4,
                                               space="PSUM"))
        tpsum = ctx.enter_context(tc.tile_pool(name="tpsum", bufs=2,
                                               space="PSUM"))

        def load_const(shape, dtp, src, tag):
            t = const.tile(shape, dtp, tag=tag)
            nc.sync.dma_start(t[:], src[:])
            return t

        w1_sb = load_const([F0, F1], dt.bfloat16, w1_d, "w1")
        w2_sb = load_const([F1, F2], dt.bfloat16, w2_d, "w2")
        b1_sb = load_const([P, F1], dt.float32, b1_d, "b1")
        b2_sb = load_const([P, F2], dt.float32, b2_d, "b2")
        dgm_sb = load_const([P, TBLK], dt.float32, dgm_d, "dgm")
        dow_sb = load_const([P, TPC], dt.float32, dow_d, "dow")
        iota_sb = load_const([P, WIN], dt.bfloat16, iota_d, "iota")
        id_sb = load_const([P, P], dt.float32, id_d, "ident")

        agg1 = persist.tile([P, TPC * F1], dt.float32, tag="agg1")
        g1_own = persist.tile([P, TPC * F1], dt.float32, tag="g1own")
        agg2 = persist.tile([P, TPC * F2], dt.float32, tag="agg2")
        g2_own = persist.tile([P, TPC * F2], dt.float32, tag="g2own")
        h1T_sb = persist.tile([F1, PER_CORE], dt.bfloat16, tag="h1T")

        table1_v = table1[:].rearrange("(p t) f -> p t f", p=P)
        table2_v = table2[:].rearrange("(p t) f -> p t f", p=P)

        def gemm_own(xo, w_sb, fout, g_own):
            for c in range(TPC):
                ps = gpsum.tile([P, fout], dt.float32, tag="gps")
                nc.tensor.matmul(
                    ps[:], lhsT=xo[:, c * P : (c + 1) * P], rhs=w_sb[:],
                    start=True, stop=True)
                nc.vector.tensor_scalar(
                    g_own[:, c * fout : (c + 1) * fout], ps[:],
                    dow_sb[:, c : c + 1], None, op0=mybir.AluOpType.mult)

        def aggregate(table, fout, agg):
            tau0 = 0
            for c in range(cfg.N_CHUNKS):
                wts = tiles_wg[c * CW : (c + 1) * CW]          # [CW, NR]
                t_c = chunk_tiles[c]
                it = idxp.tile([P, 8 * t_chunk_max], dt.int16, tag="idx")
                nc.sync.dma_start(
                    it[:, : 8 * t_c], idx_d[:, 8 * tau0 : 8 * (tau0 + t_c)])
                dp = dposp.tile([P, t_chunk_max], dt.bfloat16, tag="dpos")
                nc.sync.dma_start(dp[:, :t_c], dps_d[:, tau0 : tau0 + t_c])
                ms = msgsp.tile([P, t_chunk_max * ELEM], dt.bfloat16,
                                tag="msgs")
                gmax = int(os.environ.get("GCN_GMAX", "8"))
                off = 0
                for g in range(NR):
                    n_cg = int(np.sum(wts[:, g]))
                    while n_cg > 0:
                        n = min(n_cg, gmax)
                        nc.gpsimd.dma_gather(
                            ms[:, off * ELEM : (off + n) * ELEM].rearrange(
                                "p (t e) -> p t e", e=ELEM),
                            table[g * cfg.RS : (g + 1) * cfg.RS, :],
                            it[:, 8 * off : 8 * (off + n)],
                            n * P,
                            n * P,
                            ELEM)
                        off += n
                        n_cg -= n
                mb = membp.tile([P, t_chunk_max * WIN], dt.bfloat16, tag="memb")
                nc.vector.tensor_tensor(
                    out=mb[:, : t_c * WIN].rearrange("p (t w) -> p t w", w=WIN),
                    in0=dp[:, :t_c, None].to_broadcast([P, t_c, WIN]),
                    in1=iota_sb[:, None, :].to_broadcast([P, t_c, WIN]),
                    op=mybir.AluOpType.is_equal)
                # chunk-local tile index of (wl, g)
                goff = np.concatenate([[0], np.cumsum(wts.sum(axis=0))])
                for wl in range(CW):
                    wg = c * CW + wl
                    taus = []
                    for g in range(NR):
                        base = goff[g] + int(np.sum(wts[:wl, g]))
                        taus += list(range(base, base + int(wts[wl, g])))
                    ps = wpsum.tile([WIN, fout], dt.float32, tag="wps")
                    for j, tau in enumerate(taus):
                        nc.tensor.matmul(
                            ps[:],
                            lhsT=mb[:, tau * WIN : (tau + 1) * WIN],
                            rhs=ms[:, tau * ELEM : tau * ELEM + fout],
                            start=(j == 0), stop=(j == len(taus) - 1))
                    prow = (wg % (P // WIN)) * WIN
                    col = (wg // (P // WIN)) * fout
                    nc.vector.tensor_copy(
                        agg[prow : prow + WIN, col : col + fout], ps[:])
                tau0 += t_c

        def finale(agg, g_own, b_sb, fout, relu):
            nc.vector.tensor_add(agg[:], agg[:], g_own[:])
            a3 = agg[:].rearrange("p (c f) -> p c f", f=fout)
            nc.vector.tensor_tensor(
                out=a3, in0=a3,
                in1=dow_sb[:, :, None].to_broadcast([P, TPC, fout]),
                op=mybir.AluOpType.mult)
            nc.vector.tensor_tensor(
                out=a3, in0=a3,
                in1=b_sb[:, None, :].to_broadcast([P, TPC, fout]),
                op=mybir.AluOpType.add)
            if relu:
                nc.vector.tensor_scalar_max(agg[:], agg[:], 0.0)

        # ================= layer 1 =================
        HS = PER_CORE // 2

        def xs_get(s):
            xs = xslab.tile([F0, HS], dt.bfloat16, tag="xs")
            nc.sync.dma_start(xs[:], xT_d[:, s * HS : (s + 1) * HS])
            return xs

        # layer-1 table GEMM over 16 half-slabs
        def gemm_table_l1():
            for s in range(2 * NCORES):
                xs = xs_get(s)
                stg = None
                q0 = 0
                assert TPC % 2 == 0
                for j in range(TPC // 2):
                    if stg is None:
                        stg = stgp.tile([P, SW * ELEM], dt.bfloat16, tag="stg")
                        nc.vector.memset(stg[:], 0.0)
                        q0 = j
                    t = s * (TPC // 2) + j
                    ps = gpsum.tile([P, F1], dt.float32, tag="gps")
                    nc.tensor.matmul(
                        ps[:], lhsT=xs[:, j * P : (j + 1) * P], rhs=w1_sb[:],
                        start=True, stop=True)
                    nc.vector.tensor_scalar(
                        stg[:, (j - q0) * ELEM : (j - q0) * ELEM + F1],
                        ps[:], dgm_sb[:, t : t + 1], None,
                        op0=mybir.AluOpType.mult)
                    if j - q0 == SW - 1 or j == TPC // 2 - 1:
                        nw = j - q0 + 1
                        nc.sync.dma_start(
                            table1_v[:, t - nw + 1 : t + 1, :],
                            stg[:, : nw * ELEM].rearrange(
                                "p (t f) -> p t f", f=ELEM))
                        stg = None

        gemm_table_l1()
        xo = const.tile([F0, PER_CORE], dt.bfloat16, tag="xo")
        nc.sync.dma_start(xo[:], xTo_d[:])
        gemm_own(xo, w1_sb, F1, g1_own)
        aggregate(table1, F1, agg1)
        finale(agg1, g1_own, b1_sb, F1, relu=True)

        for cch in range(TPC):
            pt = tpsum.tile([F1, P], dt.float32, tag="tp")
            nc.tensor.transpose(
                pt[:], agg1[:, cch * F1 : (cch + 1) * F1], id_sb[:])
            nc.vector.tensor_copy(h1T_sb[:, cch * P : (cch + 1) * P], pt[:])
        nc.sync.dma_start(h1T_own_d[:], h1T_sb[:])
        nc.gpsimd.collective_compute(
            "AllGather", mybir.AluOpType.bypass,
            replica_groups=[list(range(NCORES))],
            ins=[h1T_own_d[:]], outs=[h1T_all_d[:]])

        # ================= layer 2 =================
        def gemm_table_l2():
            for s in range(2 * NCORES):
                hs = xslab.tile([F0, HS], dt.bfloat16, tag="xs")
                r, half = s // 2, s % 2
                nc.sync.dma_start(
                    hs[:F1, :],
                    h1T_all_d[r * F1 : (r + 1) * F1,
                              half * HS : (half + 1) * HS])
                stg = None
                q0 = 0
                for j in range(TPC // 2):
                    if stg is None:
                        stg = stgp.tile([P, SW * ELEM], dt.bfloat16, tag="stg")
                        nc.vector.memset(stg[:], 0.0)
                        q0 = j
                    t = s * (TPC // 2) + j
                    ps = gpsum.tile([P, F2], dt.float32, tag="gps")
                    nc.tensor.matmul(
                        ps[:], lhsT=hs[:F1, j * P : (j + 1) * P], rhs=w2_sb[:],
                        start=True, stop=True)
                    nc.vector.tensor_scalar(
                        stg[:, (j - q0) * ELEM : (j - q0) * ELEM + F2],
                        ps[:], dgm_sb[:, t : t + 1], None,
                        op0=mybir.AluOpType.mult)
                    if j - q0 == SW - 1 or j == TPC // 2 - 1:
                        nw = j - q0 + 1
                        nc.sync.dma_start(
                            table2_v[:, t - nw + 1 : t + 1, :],
                            stg[:, : nw * ELEM].rearrange(
                                "p (t f) -> p t f", f=ELEM))
                        stg = None

        gemm_table_l2()
        gemm_own(h1T_sb, w2_sb, F2, g2_own)
        aggregate(table2, F2, agg2)
        finale(agg2, g2_own, b2_sb, F2, relu=False)
        nc.sync.dma_start(out_d[:], agg2[:])

    nc.compile()
    return nc


def make_in_maps(cfg, per_core, shared, weights):
    in_maps = []
    for k in range(cfg.NCORES):
        in_maps.append({
            "xT": shared["xT"],
            "xT_own": per_core[k]["xT_own"],
            "W1": weights["W1"], "b1": weights["b1"],
            "W2": weights["W2"], "b2": weights["b2"],
            "dinv_gemm": shared["dinv_gemm"],
            "dinv_own": per_core[k]["dinv_own"],
            "idx_all": per_core[k]["idx_all"],
            "dpos_all": per_core[k]["dpos_all"],
            "iota": shared["iota"],
            "ident": shared["ident"],
        })
    return in_maps


def pack_weights(cfg, W1, b1, W2, b2):
    return dict(
        W1=np.ascontiguousarray(np.asarray(W1, np.float32).astype(BF16)),
        b1=np.ascontiguousarray(np.broadcast_to(
            np.asarray(b1, np.float32).reshape(1, cfg.F1), (P, cfg.F1))),
        W2=np.ascontiguousarray(np.asarray(W2, np.float32).astype(BF16)),
        b2=np.ascontiguousarray(np.broadcast_to(
            np.asarray(b2, np.float32).reshape(1, cfg.F2), (P, cfg.F2))),
    )


# ----------------------------------------------------------------- driver ---
def _install_trace_shim():
    """Provide antenv.axon_hooks (NTFF hook) + stub artifact upload."""
    import sys, types
    try:
        import antenv.axon_hooks  # noqa: F401
        have = True
    except ImportError:
        have = False
    if not have:
        mod = types.ModuleType("antenv.axon_hooks")
        state = {}

        def set_axon_ntff_profile_hook(h):
            state["h"] = h

        def get_axon_ntff_profile_hook():
            if "h" not in state:
                try:
                    from trn_agent_boot.trn_boot import _ntff_profile_via_ctypes
                    state["h"] = _ntff_profile_via_ctypes(
                        "/opt/axon/libaxon_pjrt.so")
                except Exception as e:
                    print("axon_hooks shim failed:", e)
                    state["h"] = None
            return state["h"]

        mod.set_axon_ntff_profile_hook = set_axon_ntff_profile_hook
        mod.get_axon_ntff_profile_hook = get_axon_ntff_profile_hook
        sys.modules["antenv.axon_hooks"] = mod
    from concourse import bass_utils
    bass_utils.upload_artifacts = lambda d: str(d)


_CACHE = {}


def kernel(x, edge_index, W1, b1, W2, b2):
    from concourse import bass_utils

    cfg = FULL
    tiles_wg, per_core, shared = prepare(cfg, x, edge_index)
    weights = pack_weights(cfg, W1, b1, W2, b2)
    key = tiles_wg.tobytes()
    if key not in _CACHE:
        _CACHE[key] = build(cfg, tiles_wg)
    nc = _CACHE[key]

    trace = bool(int(os.environ.get("GCN_TRACE", "0")))
    if trace:
        _install_trace_shim()
    in_maps = make_in_maps(cfg, per_core, shared, weights)
    res = bass_utils.run_bass_kernel_spmd(
        nc, in_maps, core_ids=list(range(cfg.NCORES)),
        trace=trace)
    outs = [r["out"] for r in res.results]
    out = unshard(cfg, outs)
    if res.exec_time_ns is not None:
        print(f"HW exec time: {res.exec_time_ns} ns")
    return out
